# revision 27
# baseline (speedup 1.0000x reference)
"""Causal MHA on 8 trn2 cores — v2b: single-phase interleaved schedule.

Sharding: 8 cores = 4 batches x 2 head-groups (8 heads each).

Schedule: proj(st=0) runs first; attention for q-tile qt is interleaved
with projection chains for s-tile st=qt+1 and the output projection of
qt-1, so the PE never waits on the ACT exp pipeline. All data bf16;
psum f32. Causal mask folded into the scores matmul group (PE). Z
reciprocal via fast DVE approx, broadcast via K=1 selector matmuls.

PSUM budget (8 banks): ss 2bufs x 2 + poA/poB 2 + chain pool 2.
"""

import sys

if "/opt/trn_rl_repo" not in sys.path:
    sys.path.insert(0, "/opt/trn_rl_repo")

import numpy as np

import concourse.bass as bass
import concourse.mybir as mybir
from concourse import bacc, tile
from concourse.bass_utils import run_bass_kernel_spmd

P = 128
D_MODEL = 1024
NUM_HEADS = 16
DK = 64
B, S = 4, 2048
HG = NUM_HEADS // 2
MG = HG * DK
N_CORES = 8

QT = S // 512
JT = S // P
KT = D_MODEL // P
MSUB = MG // P
NT = D_MODEL // P

F32 = mybir.dt.float32
F32R = mybir.dt.float32r
BF16 = mybir.dt.bfloat16
EXP = mybir.ActivationFunctionType.Exp

_CACHED_NC = None


def build_nc() -> bass.Bass:
    nc = bacc.Bacc("TRN2", target_bir_lowering=False, debug=False)

    # inputs pre-tiled host-side to partition-major layout so every DMA
    # partition-row is 8KB contiguous (8x fewer DMA packets than the
    # natural [d_model, seq] layout)
    xt4 = nc.dram_tensor("xt4", [QT, P, KT, 512], BF16, kind="ExternalInput")
    wq4 = nc.dram_tensor("wq4", [P, KT, MG], BF16, kind="ExternalInput")
    wk4 = nc.dram_tensor("wk4", [P, KT, MG], BF16, kind="ExternalInput")
    wv4 = nc.dram_tensor("wv4", [P, KT, MG], BF16, kind="ExternalInput")
    wo4 = nc.dram_tensor("wo4", [P, MSUB, D_MODEL], BF16, kind="ExternalInput")
    tmask = nc.dram_tensor("tmask", [P, P], BF16, kind="ExternalInput")
    ident2 = nc.dram_tensor("ident2", [P, 2, P], BF16, kind="ExternalInput")
    # output y^T tiled [qt, p, nt, 512]: 4 nt-tiles per DMA -> 4KB rows
    yT4 = nc.dram_tensor("yT4", [QT, P, NT, 512], BF16, kind="ExternalOutput")

    with tile.TileContext(nc) as tc:
        with (
            tc.tile_pool(name="wpool", bufs=1) as wpool,
            tc.tile_pool(name="qkv", bufs=1) as qkv,
            tc.tile_pool(name="xs", bufs=2) as xs,
            tc.tile_pool(name="oh", bufs=2) as ohp,
            tc.tile_pool(name="ys", bufs=4) as ysp,
            tc.tile_pool(name="attn", bufs=4) as attn,
            tc.tile_pool(name="attnc", bufs=1) as attnc,
            tc.tile_pool(name="ps_s", bufs=2, space="PSUM") as ps_s,
            tc.tile_pool(name="ps_o", bufs=1, space="PSUM") as ps_o,
            tc.tile_pool(name="ps_c", bufs=2, space="PSUM") as ps_c,
        ):
            # ---- persistent sbuf ----
            w_sb = {}
            for name in ("q", "k", "v"):
                w_sb[name] = wpool.tile(
                    [P, KT, MG], BF16, tag=f"w{name}", name=f"w{name}"
                )
            wo_sb = wpool.tile([P, MSUB, D_MODEL], BF16, tag="wo")
            qT_sb = qkv.tile([P, MSUB, S], BF16, tag="qT")
            kT_sb = qkv.tile([P, MSUB, S], BF16, tag="kT")
            v_sb = qkv.tile([P, JT, HG, DK + 1], BF16, tag="v")
            nc.vector.memset(v_sb[:, :, :, DK : DK + 1], 1.0)

            tm_sb = attnc.tile([P, P], BF16, tag="tm")
            id2_sb = attnc.tile([P, 2, P], BF16, tag="id2")
            # [33,128] selector: row 0 lights partitions 0:64, row 32 lights
            # 64:128 (partition bases must be 32-aligned, so Z lives in
            # partitions 0 and 32 of z2)
            sel2 = attnc.tile([33, P], F32R, tag="sel2")
            nc.vector.memset(sel2[:].bitcast(F32), 0.0)
            nc.vector.memset(sel2[0:1, 0:DK].bitcast(F32), 1.0)
            nc.vector.memset(sel2[32:33, DK:P].bitcast(F32), 1.0)

            # ---- input DMA: split across queues, first-needed-first ----
            warm_src = attnc.tile([P, 256], BF16, tag="warm_src")
            nc.vector.memset(warm_src[:], 0.5)
            nc.sync.dma_start(tm_sb[:], tmask[:])
            nc.sync.dma_start(id2_sb[:], ident2[:])
            x_tiles = [None] * QT

            def issue_x_dma(st):
                x_tiles[st] = xs.tile([P, KT, 512], BF16, tag="x", name=f"x{st}")
                for kp in range(4):
                    nc.sync.dma_start(
                        x_tiles[st][:, 2 * kp : 2 * kp + 2],
                        xt4[st, :, 2 * kp : 2 * kp + 2],
                    )

            issue_x_dma(0)
            for name, wsrc in (("q", wq4), ("k", wk4), ("v", wv4)):
                nc.sync.dma_start(w_sb[name][:, 0:4], wsrc[:, 0:4])
                nc.sync.dma_start(w_sb[name][:, 4:8], wsrc[:, 4:8])
            nc.sync.dma_start(wo_sb[:], wo4[:])

            # warm the PE while the x/w DMAs land (memset source, no DMA
            # dependency): p-state ramps before the first projection chain
            warm = ps_c.tile([P, 512], F32, tag="pp", name="warm")
            for _ in range(40):
                nc.tensor.matmul(
                    warm[:, 0:256], warm_src[:, 0:P], warm_src[:],
                    start=True, stop=True, skip_group_check=True,
                )

            # ---- filler-step factories (each step = ~4 matmuls on PE) ----
            def proj_qk_steps(name, dst, st):
                ssl = slice(st * 512, (st + 1) * 512)
                w = w_sb[name]
                x_t = x_tiles[st]
                steps = []
                for mt in range(MSUB):
                    msl = slice(mt * P, (mt + 1) * P)
                    holder = {}

                    def s1(mt=mt, msl=msl, holder=holder):
                        pt = ps_c.tile([P, 512], F32, tag="pp", name="prq")
                        holder["pt"] = pt
                        for kt in range(4):
                            nc.tensor.matmul(
                                pt[:], w[:, kt, msl], x_t[:, kt],
                                start=(kt == 0), stop=False,
                            )

                    def s2(mt=mt, msl=msl, holder=holder):
                        pt = holder["pt"]
                        for kt in range(4, KT):
                            nc.tensor.matmul(
                                pt[:], w[:, kt, msl], x_t[:, kt],
                                start=False, stop=(kt == KT - 1),
                            )
                        nc.vector.tensor_copy(dst[:, mt, ssl], pt[:])

                    steps += [s1, s2]
                return steps

            def proj_v_steps(st):
                x_t = x_tiles[st]
                steps = []
                for ssub in range(4):
                    jt = st * 4 + ssub
                    s0 = ssub * P
                    holder = {}

                    def s1(jt=jt, s0=s0, holder=holder):
                        pt = ps_c.tile([P, 512], F32, tag="pp", name="prv")
                        holder["pt"] = pt
                        for kt in range(4):
                            nc.tensor.matmul(
                                pt[:], x_t[:, kt, s0 : s0 + P], w_sb["v"][:, kt],
                                start=(kt == 0), stop=False,
                            )

                    def s2(jt=jt, s0=s0, holder=holder):
                        pt = holder["pt"]
                        for kt in range(4, KT):
                            nc.tensor.matmul(
                                pt[:], x_t[:, kt, s0 : s0 + P], w_sb["v"][:, kt],
                                start=False, stop=(kt == KT - 1),
                            )
                        nc.vector.tensor_copy(
                            v_sb[:, jt, :, 0:DK],
                            pt.rearrange("p (h d) -> p h d", h=HG),
                        )

                    steps += [s1, s2]
                return steps

            def outproj_steps(ohT_prev, qt_prev):
                steps = []
                holder = {}
                for nt in range(NT):
                    def s1(nt=nt):
                        py = ps_c.tile([P, 512], F32, tag="pp", name="py")
                        for mt in range(MSUB):
                            nc.tensor.matmul(
                                py[:],
                                wo_sb[:, mt, nt * P : (nt + 1) * P],
                                ohT_prev[:, mt, :],
                                start=(mt == 0), stop=(mt == MSUB - 1),
                            )
                        if nt % 4 == 0:
                            holder["y4"] = ysp.tile(
                                [P, 4, 512], BF16, tag="y", name="y4"
                            )
                        nc.vector.tensor_copy(holder["y4"][:, nt % 4, :], py[:])
                        if nt % 4 == 3:  # 4 tiles buffered -> one 4KB-row DMA
                            nc.sync.dma_start(
                                yT4[qt_prev, :, nt - 3 : nt + 1], holder["y4"][:]
                            )

                    steps.append(s1)
                return steps

            # ---- attention primitives ----
            def emit_scores(qt, hp, jt):
                jsl = slice(jt * P, (jt + 1) * P)
                di = jt - qt * 4
                delta = 128 * di if di >= 0 else 0
                qsl_d = slice(qt * 512 + delta, (qt + 1) * 512)
                ss = ps_s.tile([P, 2, 512], F32, tag="ss")
                nc.tensor.matmul(
                    ss[:, 0, delta:],
                    kT_sb[0:DK, hp, jsl],
                    qT_sb[0:DK, hp, qsl_d],
                    start=True, stop=False, skip_group_check=True,
                )
                nc.tensor.matmul(
                    ss[:, 1, delta:],
                    kT_sb[DK:P, hp, jsl],
                    qT_sb[DK:P, hp, qsl_d],
                    start=True, stop=(di < 0), skip_group_check=True,
                )
                if di >= 0:
                    nc.tensor.matmul(
                        ss[:, :, delta : delta + P],
                        tm_sb[:], id2_sb[:],
                        start=False, stop=True, skip_group_check=True,
                    )
                return ss, delta

            # ---- main interleaved schedule ----
            # x(st=1) streams while proj(st=0) runs standalone (attention
            # qt=0 depends on proj(st=0))
            issue_x_dma(1)
            for step in proj_qk_steps("q", qT_sb, 0):
                step()
            for step in proj_qk_steps("k", kT_sb, 0):
                step()
            for step in proj_v_steps(0):
                step()

            prev = None  # (ohT, qt) with outproj pending
            deferred_kv3 = None
            fin = {}
            for qt in range(QT):
                # x DMA two q-tiles ahead (xs bufs=2: the tile waits for the
                # previous generation's readers automatically)
                if qt + 2 < QT:
                    issue_x_dma(qt + 2)
                fillers = []
                nfront = 0
                if qt + 1 < QT:
                    st = qt + 1
                    fillers += proj_qk_steps("q", qT_sb, st)
                    if st < QT - 1:
                        fillers += proj_qk_steps("k", kT_sb, st)
                        fillers += proj_v_steps(st)
                    else:
                        # defer k/v(st=3) into qt=3's early blocks: qt3 is
                        # ACT(exp)-bound, so this PE work fills its bubbles
                        deferred_kv3 = (
                            proj_qk_steps("k", kT_sb, st) + proj_v_steps(st)
                        )
                if qt == QT - 1 and deferred_kv3 is not None:
                    fillers = deferred_kv3 + fillers
                    # k/v(st3) feeds this qt's jt>=12 blocks: hp0 hits jt=12
                    # at block 12, so these steps must front-load
                    nfront = len(deferred_kv3)
                if prev is not None:
                    fillers += outproj_steps(prev[0], prev[1])

                njt = 4 * (qt + 1)
                ohT = ohp.tile([P, MSUB, 512], BF16, tag="ohT")
                work = [(hp, jt) for hp in range(MSUB) for jt in range(njt)]
                nw = len(work)
                nf = len(fillers)
                fi = 0
                po = {}
                pend = emit_scores(qt, 0, 0)
                for wi, (hp, jt) in enumerate(work):
                    if jt == 0:
                        po[hp] = (
                            ps_o.tile([DK + 1, 512], F32, tag="poA", name="po_a"),
                            ps_o.tile([DK + 1, 512], F32, tag="poB", name="po_b"),
                        )
                    ss, delta = pend
                    pp = attn.tile([P, 2, 512], BF16, tag="pp")
                    nc.scalar.activation(
                        pp[:, :, delta:], ss[:, :, delta:], EXP, scale=0.125
                    )
                    # next block's scores ahead of this block's PV
                    if wi + 1 < nw:
                        pend = emit_scores(qt, *work[wi + 1])
                    # filler quota for this block (front-loaded steps must
                    # finish within the first 11 blocks); blocks 0-1 pull
                    # extra so the PE stays busy while the scores->exp->PV
                    # pipeline refills at the q-tile boundary
                    want = (wi + 1) * nf // nw
                    if wi < 2:
                        want = max(want, min(nf, 2 * (wi + 1)))
                    if nfront and wi < 12:
                        want = max(want, min(nfront, (wi + 1) * nfront // 11))
                    while fi < want:
                        fillers[fi]()
                        fi += 1
                    po_a, po_b = po[hp]
                    nc.tensor.matmul(
                        po_a[:, delta:],
                        v_sb[:, jt, 2 * hp, :],
                        pp[:, 0, delta:],
                        start=(jt == 0), stop=(jt == njt - 1),
                    )
                    nc.tensor.matmul(
                        po_b[:, delta:],
                        v_sb[:, jt, 2 * hp + 1, :],
                        pp[:, 1, delta:],
                        start=(jt == 0), stop=(jt == njt - 1),
                    )
                    if jt == njt - 1:
                        z2 = attnc.tile([33, 512], F32R, tag="z2")
                        if hp == 0 and qt == 0:
                            # rows 1..31 are never written; zero once so the
                            # K=33 broadcast matmul multiplies 0 * 0
                            nc.vector.memset(z2[:].bitcast(F32), 0.0)
                        dst = ohT[:, hp, :]
                        nc.vector.tensor_copy(z2[0:1, :], po_a[DK : DK + 1, :])
                        nc.vector.tensor_copy(z2[32:33, :], po_b[DK : DK + 1, :])
                        nc.vector.tensor_copy(dst[0:DK], po_a[0:DK, :])
                        nc.vector.tensor_copy(dst[DK:P], po_b[0:DK, :])
                        last_ep = qt == QT - 1 and hp == MSUB - 1
                        if last_ep:
                            # very last epilogue: pre-open the first final
                            # outproj chain (mt 0..2 only need already-scaled
                            # heads) so the PE covers the z2->bcz->mult chain
                            fin["py0"] = fin_py0 = ps_c.tile(
                                [P, 512], F32, tag="pp", name="py0f"
                            )
                            for mt in range(MSUB - 1):
                                nc.tensor.matmul(
                                    fin_py0[:],
                                    wo_sb[:, mt, 0:P],
                                    ohT[:, mt, :],
                                    start=(mt == 0), stop=False,
                                )
                        # keep the PE busy while the DVE drains z2 (the bcz
                        # matmul below would otherwise head-of-line block)
                        got = 0
                        for _ in range(2):
                            if fi < nf:
                                fillers[fi]()
                                fi += 1
                                got += 1
                        bcz = ps_c.tile([P, 512], F32, tag="pp", name="bcz")
                        if not last_ep:
                            # no real filler left: dependency-free dummy
                            # matmuls bridge the z2-copy latency instead of
                            # letting the PE idle (and drop p-state)
                            for _ in range((2 - got) * 6):
                                nc.tensor.matmul(
                                    bcz[:, 0:256], warm_src[:, 0:P],
                                    warm_src[:], start=True, stop=True,
                                    skip_group_check=True,
                                )
                        nc.tensor.matmul(
                            bcz[:], sel2[:], z2[:], start=True, stop=True
                        )
                        bcr = attnc.tile([P, 512], F32, tag="bcr")
                        nc.vector.reciprocal_approx_fast(bcr[:], bcz[:])
                        nc.gpsimd.tensor_tensor(
                            dst, dst, bcr[:], mybir.AluOpType.mult
                        )
                        if last_ep:
                            nc.tensor.matmul(
                                fin_py0[:],
                                wo_sb[:, MSUB - 1, 0:P],
                                ohT[:, MSUB - 1, :],
                                start=False, stop=True,
                            )
                while fi < len(fillers):
                    fillers[fi]()
                    fi += 1
                prev = (ohT, qt)
            # final output projection: nt=0 was computed inside the last
            # epilogue; evict it and run nt 1..7
            ohT3, qt3 = prev
            y2f = None
            for nt in range(NT):
                if nt % 2 == 0:
                    y2f = ysp.tile([P, 2, 512], BF16, tag="y2", name="y2f")
                if nt == 0:
                    py = fin["py0"]
                else:
                    py = ps_c.tile([P, 512], F32, tag="pp", name="pyf")
                    for mt in range(MSUB):
                        nc.tensor.matmul(
                            py[:],
                            wo_sb[:, mt, nt * P : (nt + 1) * P],
                            ohT3[:, mt, :],
                            start=(mt == 0), stop=(mt == MSUB - 1),
                        )
                # alternate eviction engines so chains never wait on one
                if nt % 2 == 0:
                    nc.vector.tensor_copy(y2f[:, 0, :], py[:])
                else:
                    nc.scalar.copy(y2f[:, 1, :], py[:])
                    nc.sync.dma_start(yT4[qt3, :, nt - 1 : nt + 1], y2f[:])

    nc.finalize()
    return nc


def _get_nc() -> bass.Bass:
    global _CACHED_NC
    if _CACHED_NC is None:
        _CACHED_NC = build_nc()
    return _CACHED_NC


def _make_masks() -> np.ndarray:
    import ml_dtypes

    k = np.arange(P)[:, None]
    j = np.arange(P)[None, :]
    return np.where(j > k, -30000.0, 0.0).astype(ml_dtypes.bfloat16)


def _make_ident2() -> np.ndarray:
    import ml_dtypes

    e = np.eye(P, dtype=np.float32)
    return np.stack([e, e], axis=1).astype(ml_dtypes.bfloat16)


def make_in_maps(inputs):
    import ml_dtypes

    bf = ml_dtypes.bfloat16
    x = np.asarray(inputs["x"], np.float32)
    q_heads = np.asarray(inputs["q_heads"], np.float32)
    k_heads = np.asarray(inputs["k_heads"], np.float32)
    v_heads = np.asarray(inputs["v_heads"], np.float32)
    output_proj = np.asarray(inputs["output_proj"], np.float32)

    tm = _make_masks()
    id2 = _make_ident2()

    def tile_w(wT):  # [1024, 512] -> [p, kt, m]
        return np.ascontiguousarray(
            wT.reshape(KT, P, MG).transpose(1, 0, 2)
        ).astype(bf)

    in_maps = []
    for core in range(N_CORES):
        b, g = divmod(core, 2)
        gsl = slice(g * MG, (g + 1) * MG)
        xT = x[b].T  # [1024, 2048]
        xt4 = np.ascontiguousarray(
            xT.reshape(KT, P, QT, 512).transpose(2, 1, 0, 3)
        ).astype(bf)  # [st, p, kt, 512]
        wo = output_proj[:, gsl].T  # [512, 1024]
        wo4 = np.ascontiguousarray(
            wo.reshape(MSUB, P, D_MODEL).transpose(1, 0, 2)
        ).astype(bf)
        in_maps.append(
            {
                "xt4": xt4,
                "wq4": tile_w(q_heads[gsl].T),
                "wk4": tile_w(k_heads[gsl].T),
                "wv4": tile_w(v_heads[gsl].T),
                "wo4": wo4,
                "tmask": tm,
                "ident2": id2,
            }
        )
    return in_maps


def kernel(x, q_heads, k_heads, v_heads, output_proj):
    inputs = {
        "x": x,
        "q_heads": q_heads,
        "k_heads": k_heads,
        "v_heads": v_heads,
        "output_proj": output_proj,
    }
    in_maps = make_in_maps(inputs)
    nc = _get_nc()
    res = run_bass_kernel_spmd(nc, in_maps, list(range(N_CORES)))
    y = np.empty((B, S, D_MODEL), np.float32)
    for b in range(B):
        # yT4 [qt, p, nt, 512] -> yT [nt*128+p, qt*512+c]
        acc = res.results[2 * b]["yT4"].astype(np.float32) + res.results[
            2 * b + 1
        ]["yT4"].astype(np.float32)
        yT = acc.transpose(2, 1, 0, 3).reshape(D_MODEL, S)
        y[b] = yT.T
    return y


# revision 29
# speedup vs baseline: 1.0025x; 1.0025x over previous
"""Causal MHA on 8 trn2 cores — v2b: single-phase interleaved schedule.

Sharding: 8 cores = 4 batches x 2 head-groups (8 heads each).

Schedule: proj(st=0) runs first; attention for q-tile qt is interleaved
with projection chains for s-tile st=qt+1 and the output projection of
qt-1, so the PE never waits on the ACT exp pipeline. All data bf16;
psum f32. Causal mask folded into the scores matmul group (PE). Z
reciprocal via fast DVE approx, broadcast via K=1 selector matmuls.

PSUM budget (8 banks): ss 2bufs x 2 + poA/poB 2 + chain pool 2.
"""

import sys

if "/opt/trn_rl_repo" not in sys.path:
    sys.path.insert(0, "/opt/trn_rl_repo")

import numpy as np

import concourse.bass as bass
import concourse.mybir as mybir
from concourse import bacc, tile
from concourse.bass_utils import run_bass_kernel_spmd

P = 128
D_MODEL = 1024
NUM_HEADS = 16
DK = 64
B, S = 4, 2048
HG = NUM_HEADS // 2
MG = HG * DK
N_CORES = 8

QT = S // 512
JT = S // P
KT = D_MODEL // P
MSUB = MG // P
NT = D_MODEL // P

F32 = mybir.dt.float32
F32R = mybir.dt.float32r
BF16 = mybir.dt.bfloat16
EXP = mybir.ActivationFunctionType.Exp

_CACHED_NC = None


def build_nc() -> bass.Bass:
    nc = bacc.Bacc("TRN2", target_bir_lowering=False, debug=False)

    # inputs pre-tiled host-side to partition-major layout so every DMA
    # partition-row is 8KB contiguous (8x fewer DMA packets than the
    # natural [d_model, seq] layout)
    xt4 = nc.dram_tensor("xt4", [QT, P, KT, 512], BF16, kind="ExternalInput")
    wq4 = nc.dram_tensor("wq4", [P, KT, MG], BF16, kind="ExternalInput")
    wk4 = nc.dram_tensor("wk4", [P, KT, MG], BF16, kind="ExternalInput")
    wv4 = nc.dram_tensor("wv4", [P, KT, MG], BF16, kind="ExternalInput")
    wo4 = nc.dram_tensor("wo4", [P, MSUB, D_MODEL], BF16, kind="ExternalInput")
    tmask = nc.dram_tensor("tmask", [P, P], BF16, kind="ExternalInput")
    ident2 = nc.dram_tensor("ident2", [P, 2, P], BF16, kind="ExternalInput")
    # output y^T tiled [qt, p, nt, 512]: 4 nt-tiles per DMA -> 4KB rows
    yT4 = nc.dram_tensor("yT4", [QT, P, NT, 512], BF16, kind="ExternalOutput")

    with tile.TileContext(nc) as tc:
        with (
            tc.tile_pool(name="wpool", bufs=1) as wpool,
            tc.tile_pool(name="qkv", bufs=1) as qkv,
            tc.tile_pool(name="xs", bufs=2) as xs,
            tc.tile_pool(name="oh", bufs=2) as ohp,
            tc.tile_pool(name="ys", bufs=4) as ysp,
            tc.tile_pool(name="attn", bufs=3) as attn,
            tc.tile_pool(name="attnc", bufs=1) as attnc,
            tc.tile_pool(name="ps_s", bufs=2, space="PSUM") as ps_s,
            tc.tile_pool(name="ps_o", bufs=1, space="PSUM") as ps_o,
            tc.tile_pool(name="ps_c", bufs=2, space="PSUM") as ps_c,
        ):
            # ---- persistent sbuf ----
            w_sb = {}
            for name in ("q", "k", "v"):
                w_sb[name] = wpool.tile(
                    [P, KT, MG], BF16, tag=f"w{name}", name=f"w{name}"
                )
            wo_sb = wpool.tile([P, MSUB, D_MODEL], BF16, tag="wo")
            qT_sb = qkv.tile([P, MSUB, S], BF16, tag="qT")
            kT_sb = qkv.tile([P, MSUB, S], BF16, tag="kT")
            v_sb = qkv.tile([P, JT, HG, DK + 1], BF16, tag="v")
            nc.vector.memset(v_sb[:, :, :, DK : DK + 1], 1.0)

            tm_sb = attnc.tile([P, P], BF16, tag="tm")
            id2_sb = attnc.tile([P, 2, P], BF16, tag="id2")
            # [33,128] selector: row 0 lights partitions 0:64, row 32 lights
            # 64:128 (partition bases must be 32-aligned, so Z lives in
            # partitions 0 and 32 of z2)
            sel2 = attnc.tile([33, P], F32R, tag="sel2")
            nc.vector.memset(sel2[:].bitcast(F32), 0.0)
            nc.vector.memset(sel2[0:1, 0:DK].bitcast(F32), 1.0)
            nc.vector.memset(sel2[32:33, DK:P].bitcast(F32), 1.0)

            # ---- input DMA: split across queues, first-needed-first ----
            warm_src = attnc.tile([P, 256], BF16, tag="warm_src")
            nc.vector.memset(warm_src[:], 0.5)
            nc.sync.dma_start(tm_sb[:], tmask[:])
            nc.sync.dma_start(id2_sb[:], ident2[:])
            x_tiles = [None] * QT

            def issue_x_dma(st):
                x_tiles[st] = xs.tile([P, KT, 512], BF16, tag="x", name=f"x{st}")
                for kp in range(4):
                    nc.sync.dma_start(
                        x_tiles[st][:, 2 * kp : 2 * kp + 2],
                        xt4[st, :, 2 * kp : 2 * kp + 2],
                    )

            issue_x_dma(0)
            for name, wsrc in (("q", wq4), ("k", wk4), ("v", wv4)):
                nc.sync.dma_start(w_sb[name][:, 0:4], wsrc[:, 0:4])
                nc.sync.dma_start(w_sb[name][:, 4:8], wsrc[:, 4:8])
            nc.sync.dma_start(wo_sb[:], wo4[:])

            # warm the PE while the x/w DMAs land (memset source, no DMA
            # dependency): p-state ramps before the first projection chain
            warm = ps_c.tile([P, 512], F32, tag="pp", name="warm")
            for _ in range(50):
                nc.tensor.matmul(
                    warm[:, 0:256], warm_src[:, 0:P], warm_src[:],
                    start=True, stop=True, skip_group_check=True,
                )

            # ---- filler-step factories (each step = ~4 matmuls on PE) ----
            def proj_qk_steps(name, dst, st):
                ssl = slice(st * 512, (st + 1) * 512)
                w = w_sb[name]
                x_t = x_tiles[st]
                steps = []
                for mt in range(MSUB):
                    msl = slice(mt * P, (mt + 1) * P)
                    holder = {}

                    def s1(mt=mt, msl=msl, holder=holder):
                        pt = ps_c.tile([P, 512], F32, tag="pp", name="prq")
                        holder["pt"] = pt
                        for kt in range(4):
                            nc.tensor.matmul(
                                pt[:], w[:, kt, msl], x_t[:, kt],
                                start=(kt == 0), stop=False,
                            )

                    def s2(mt=mt, msl=msl, holder=holder):
                        pt = holder["pt"]
                        for kt in range(4, KT):
                            nc.tensor.matmul(
                                pt[:], w[:, kt, msl], x_t[:, kt],
                                start=False, stop=(kt == KT - 1),
                            )
                        nc.vector.tensor_copy(dst[:, mt, ssl], pt[:])

                    steps += [s1, s2]
                return steps

            def proj_v_steps(st):
                x_t = x_tiles[st]
                steps = []
                for ssub in range(4):
                    jt = st * 4 + ssub
                    s0 = ssub * P
                    holder = {}

                    def s1(jt=jt, s0=s0, holder=holder):
                        pt = ps_c.tile([P, 512], F32, tag="pp", name="prv")
                        holder["pt"] = pt
                        for kt in range(4):
                            nc.tensor.matmul(
                                pt[:], x_t[:, kt, s0 : s0 + P], w_sb["v"][:, kt],
                                start=(kt == 0), stop=False,
                            )

                    def s2(jt=jt, s0=s0, holder=holder):
                        pt = holder["pt"]
                        for kt in range(4, KT):
                            nc.tensor.matmul(
                                pt[:], x_t[:, kt, s0 : s0 + P], w_sb["v"][:, kt],
                                start=False, stop=(kt == KT - 1),
                            )
                        nc.vector.tensor_copy(
                            v_sb[:, jt, :, 0:DK],
                            pt.rearrange("p (h d) -> p h d", h=HG),
                        )

                    steps += [s1, s2]
                return steps

            def outproj_steps(ohT_prev, qt_prev):
                steps = []
                holder = {}
                for nt in range(NT):
                    def s1(nt=nt):
                        py = ps_c.tile([P, 512], F32, tag="pp", name="py")
                        for mt in range(MSUB):
                            nc.tensor.matmul(
                                py[:],
                                wo_sb[:, mt, nt * P : (nt + 1) * P],
                                ohT_prev[:, mt, :],
                                start=(mt == 0), stop=(mt == MSUB - 1),
                            )
                        if nt % 4 == 0:
                            holder["y4"] = ysp.tile(
                                [P, 4, 512], BF16, tag="y", name="y4"
                            )
                        nc.vector.tensor_copy(holder["y4"][:, nt % 4, :], py[:])
                        if nt % 4 == 3:  # 4 tiles buffered -> one 4KB-row DMA
                            nc.sync.dma_start(
                                yT4[qt_prev, :, nt - 3 : nt + 1], holder["y4"][:]
                            )

                    steps.append(s1)
                return steps

            # ---- attention primitives ----
            def emit_scores(qt, hp, jt):
                jsl = slice(jt * P, (jt + 1) * P)
                di = jt - qt * 4
                delta = 128 * di if di >= 0 else 0
                qsl_d = slice(qt * 512 + delta, (qt + 1) * 512)
                ss = ps_s.tile([P, 2, 512], F32, tag="ss")
                nc.tensor.matmul(
                    ss[:, 0, delta:],
                    kT_sb[0:DK, hp, jsl],
                    qT_sb[0:DK, hp, qsl_d],
                    start=True, stop=False, skip_group_check=True,
                )
                nc.tensor.matmul(
                    ss[:, 1, delta:],
                    kT_sb[DK:P, hp, jsl],
                    qT_sb[DK:P, hp, qsl_d],
                    start=True, stop=(di < 0), skip_group_check=True,
                )
                if di >= 0:
                    nc.tensor.matmul(
                        ss[:, :, delta : delta + P],
                        tm_sb[:], id2_sb[:],
                        start=False, stop=True, skip_group_check=True,
                    )
                return ss, delta

            # ---- main interleaved schedule ----
            # x(st=1) streams while proj(st=0) runs standalone (attention
            # qt=0 depends on proj(st=0))
            issue_x_dma(1)
            for step in proj_qk_steps("q", qT_sb, 0):
                step()
            for step in proj_qk_steps("k", kT_sb, 0):
                step()
            for step in proj_v_steps(0):
                step()

            prev = None  # (ohT, qt) with outproj pending
            deferred_kv3 = None
            fin = {}
            for qt in range(QT):
                # x DMA two q-tiles ahead (xs bufs=2: the tile waits for the
                # previous generation's readers automatically)
                if qt + 2 < QT:
                    issue_x_dma(qt + 2)
                fillers = []
                nfront = 0
                if qt + 1 < QT:
                    st = qt + 1
                    fillers += proj_qk_steps("q", qT_sb, st)
                    if st < QT - 1:
                        fillers += proj_qk_steps("k", kT_sb, st)
                        fillers += proj_v_steps(st)
                    else:
                        # defer k/v(st=3) into qt=3's early blocks: qt3 is
                        # ACT(exp)-bound, so this PE work fills its bubbles
                        deferred_kv3 = (
                            proj_qk_steps("k", kT_sb, st) + proj_v_steps(st)
                        )
                if qt == QT - 1 and deferred_kv3 is not None:
                    fillers = deferred_kv3 + fillers
                    # k/v(st3) feeds this qt's jt>=12 blocks: hp0 hits jt=12
                    # at block 12, so these steps must front-load
                    nfront = len(deferred_kv3)
                if prev is not None:
                    fillers += outproj_steps(prev[0], prev[1])

                njt = 4 * (qt + 1)
                ohT = ohp.tile([P, MSUB, 512], BF16, tag="ohT")
                work = [(hp, jt) for hp in range(MSUB) for jt in range(njt)]
                nw = len(work)
                nf = len(fillers)
                fi = 0
                po = {}
                pend = emit_scores(qt, 0, 0)
                for wi, (hp, jt) in enumerate(work):
                    if jt == 0:
                        po[hp] = (
                            ps_o.tile([DK + 1, 512], F32, tag="poA", name="po_a"),
                            ps_o.tile([DK + 1, 512], F32, tag="poB", name="po_b"),
                        )
                    ss, delta = pend
                    pp = attn.tile([P, 2, 512], BF16, tag="pp")
                    nc.scalar.activation(
                        pp[:, :, delta:], ss[:, :, delta:], EXP, scale=0.125
                    )
                    # next block's scores ahead of this block's PV
                    if wi + 1 < nw:
                        pend = emit_scores(qt, *work[wi + 1])
                    # filler quota for this block (front-loaded steps must
                    # finish within the first 11 blocks)
                    want = (wi + 1) * nf // nw
                    if nfront and wi < 12:
                        want = max(want, min(nfront, (wi + 1) * nfront // 11))
                    while fi < want:
                        fillers[fi]()
                        fi += 1
                    po_a, po_b = po[hp]
                    nc.tensor.matmul(
                        po_a[:, delta:],
                        v_sb[:, jt, 2 * hp, :],
                        pp[:, 0, delta:],
                        start=(jt == 0), stop=(jt == njt - 1),
                    )
                    nc.tensor.matmul(
                        po_b[:, delta:],
                        v_sb[:, jt, 2 * hp + 1, :],
                        pp[:, 1, delta:],
                        start=(jt == 0), stop=(jt == njt - 1),
                    )
                    if jt == njt - 1:
                        z2 = attnc.tile([33, 512], F32R, tag="z2")
                        if hp == 0 and qt == 0:
                            # rows 1..31 are never written; zero once so the
                            # K=33 broadcast matmul multiplies 0 * 0
                            nc.vector.memset(z2[:].bitcast(F32), 0.0)
                        dst = ohT[:, hp, :]
                        last_ep = qt == QT - 1 and hp == MSUB - 1
                        nc.vector.tensor_copy(z2[0:1, :], po_a[DK : DK + 1, :])
                        nc.vector.tensor_copy(z2[32:33, :], po_b[DK : DK + 1, :])
                        if last_ep:
                            # last epilogue: dst drains go to the (now idle)
                            # scalar engine so the DVE reaches the reciprocal
                            # ~2x sooner, and the first TWO final outproj
                            # chains pre-open (mt 0..2 need only already-
                            # scaled heads) to cover the z2->bcz->mult chain
                            nc.scalar.copy(dst[0:DK], po_a[0:DK, :])
                            nc.scalar.copy(dst[DK:P], po_b[0:DK, :])
                            for ntf in range(2):
                                pyf = ps_c.tile(
                                    [P, 512], F32, tag="pp", name=f"py{ntf}f"
                                )
                                fin[ntf] = pyf
                                for mt in range(MSUB - 1):
                                    nc.tensor.matmul(
                                        pyf[:],
                                        wo_sb[:, mt, ntf * P : (ntf + 1) * P],
                                        ohT[:, mt, :],
                                        start=(mt == 0), stop=False,
                                    )
                        else:
                            nc.vector.tensor_copy(dst[0:DK], po_a[0:DK, :])
                            nc.vector.tensor_copy(dst[DK:P], po_b[0:DK, :])
                        # keep the PE busy while the DVE drains z2 (the bcz
                        # matmul below would otherwise head-of-line block)
                        for _ in range(2):
                            if fi < nf:
                                fillers[fi]()
                                fi += 1
                        if last_ep:
                            # attention is done: borrow a free scores bank so
                            # both chain-pool bufs stay with the fin chains
                            bcz_t = ps_s.tile(
                                [P, 2, 512], F32, tag="ss", name="bcz_last"
                            )
                            bcz = bcz_t[:, 0, :]
                        else:
                            bcz = ps_c.tile([P, 512], F32, tag="pp", name="bcz")
                        nc.tensor.matmul(
                            bcz[:], sel2[:], z2[:], start=True, stop=True
                        )
                        bcr = attnc.tile([P, 512], F32, tag="bcr")
                        nc.vector.reciprocal_approx_fast(bcr[:], bcz[:])
                        nc.gpsimd.tensor_tensor(
                            dst, dst, bcr[:], mybir.AluOpType.mult
                        )
                        if last_ep:
                            for ntf in range(2):
                                nc.tensor.matmul(
                                    fin[ntf][:],
                                    wo_sb[:, MSUB - 1, ntf * P : (ntf + 1) * P],
                                    ohT[:, MSUB - 1, :],
                                    start=False, stop=True,
                                )
                while fi < len(fillers):
                    fillers[fi]()
                    fi += 1
                prev = (ohT, qt)
            # final output projection: nt=0 was computed inside the last
            # epilogue; evict it and run nt 1..7
            ohT3, qt3 = prev
            y2f = None
            for nt in range(NT):
                if nt % 2 == 0:
                    y2f = ysp.tile([P, 2, 512], BF16, tag="y2", name="y2f")
                if nt < 2:
                    py = fin[nt]
                else:
                    py = ps_c.tile([P, 512], F32, tag="pp", name="pyf")
                    for mt in range(MSUB):
                        nc.tensor.matmul(
                            py[:],
                            wo_sb[:, mt, nt * P : (nt + 1) * P],
                            ohT3[:, mt, :],
                            start=(mt == 0), stop=(mt == MSUB - 1),
                        )
                # alternate eviction engines so chains never wait on one
                if nt % 2 == 0:
                    nc.vector.tensor_copy(y2f[:, 0, :], py[:])
                else:
                    nc.scalar.copy(y2f[:, 1, :], py[:])
                    nc.sync.dma_start(yT4[qt3, :, nt - 1 : nt + 1], y2f[:])

    nc.finalize()
    return nc


def _get_nc() -> bass.Bass:
    global _CACHED_NC
    if _CACHED_NC is None:
        _CACHED_NC = build_nc()
    return _CACHED_NC


def _make_masks() -> np.ndarray:
    import ml_dtypes

    k = np.arange(P)[:, None]
    j = np.arange(P)[None, :]
    return np.where(j > k, -30000.0, 0.0).astype(ml_dtypes.bfloat16)


def _make_ident2() -> np.ndarray:
    import ml_dtypes

    e = np.eye(P, dtype=np.float32)
    return np.stack([e, e], axis=1).astype(ml_dtypes.bfloat16)


def make_in_maps(inputs):
    import ml_dtypes

    bf = ml_dtypes.bfloat16
    x = np.asarray(inputs["x"], np.float32)
    q_heads = np.asarray(inputs["q_heads"], np.float32)
    k_heads = np.asarray(inputs["k_heads"], np.float32)
    v_heads = np.asarray(inputs["v_heads"], np.float32)
    output_proj = np.asarray(inputs["output_proj"], np.float32)

    tm = _make_masks()
    id2 = _make_ident2()

    def tile_w(wT):  # [1024, 512] -> [p, kt, m]
        return np.ascontiguousarray(
            wT.reshape(KT, P, MG).transpose(1, 0, 2)
        ).astype(bf)

    in_maps = []
    for core in range(N_CORES):
        b, g = divmod(core, 2)
        gsl = slice(g * MG, (g + 1) * MG)
        xT = x[b].T  # [1024, 2048]
        xt4 = np.ascontiguousarray(
            xT.reshape(KT, P, QT, 512).transpose(2, 1, 0, 3)
        ).astype(bf)  # [st, p, kt, 512]
        wo = output_proj[:, gsl].T  # [512, 1024]
        wo4 = np.ascontiguousarray(
            wo.reshape(MSUB, P, D_MODEL).transpose(1, 0, 2)
        ).astype(bf)
        in_maps.append(
            {
                "xt4": xt4,
                "wq4": tile_w(q_heads[gsl].T),
                "wk4": tile_w(k_heads[gsl].T),
                "wv4": tile_w(v_heads[gsl].T),
                "wo4": wo4,
                "tmask": tm,
                "ident2": id2,
            }
        )
    return in_maps


def kernel(x, q_heads, k_heads, v_heads, output_proj):
    inputs = {
        "x": x,
        "q_heads": q_heads,
        "k_heads": k_heads,
        "v_heads": v_heads,
        "output_proj": output_proj,
    }
    in_maps = make_in_maps(inputs)
    nc = _get_nc()
    res = run_bass_kernel_spmd(nc, in_maps, list(range(N_CORES)))
    y = np.empty((B, S, D_MODEL), np.float32)
    for b in range(B):
        # yT4 [qt, p, nt, 512] -> yT [nt*128+p, qt*512+c]
        acc = res.results[2 * b]["yT4"].astype(np.float32) + res.results[
            2 * b + 1
        ]["yT4"].astype(np.float32)
        yT = acc.transpose(2, 1, 0, 3).reshape(D_MODEL, S)
        y[b] = yT.T
    return y


# revision 31
# speedup vs baseline: 1.0145x; 1.0120x over previous
"""Causal MHA on 8 trn2 cores — v2b: single-phase interleaved schedule.

Sharding: 8 cores = 4 batches x 2 head-groups (8 heads each).

Schedule: proj(st=0) runs first; attention for q-tile qt is interleaved
with projection chains for s-tile st=qt+1 and the output projection of
qt-1, so the PE never waits on the ACT exp pipeline. All data bf16;
psum f32. Causal mask folded into the scores matmul group (PE). Z
reciprocal via fast DVE approx, broadcast via K=1 selector matmuls.

PSUM budget (8 banks): ss 2bufs x 2 + poA/poB 2 + chain pool 2.
"""

import sys

if "/opt/trn_rl_repo" not in sys.path:
    sys.path.insert(0, "/opt/trn_rl_repo")

import numpy as np

import concourse.bass as bass
import concourse.mybir as mybir
from concourse import bacc, tile
from concourse.bass_utils import run_bass_kernel_spmd

P = 128
D_MODEL = 1024
NUM_HEADS = 16
DK = 64
B, S = 4, 2048
HG = NUM_HEADS // 2
MG = HG * DK
N_CORES = 8

QT = S // 512
JT = S // P
KT = D_MODEL // P
MSUB = MG // P
NT = D_MODEL // P

F32 = mybir.dt.float32
F32R = mybir.dt.float32r
BF16 = mybir.dt.bfloat16
EXP = mybir.ActivationFunctionType.Exp

_CACHED_NC = None


def build_nc() -> bass.Bass:
    nc = bacc.Bacc("TRN2", target_bir_lowering=False, debug=False)

    # inputs pre-tiled host-side to partition-major layout so every DMA
    # partition-row is 8KB contiguous (8x fewer DMA packets than the
    # natural [d_model, seq] layout)
    xt4 = nc.dram_tensor("xt4", [QT, P, KT, 512], BF16, kind="ExternalInput")
    wq4 = nc.dram_tensor("wq4", [P, KT, MG], BF16, kind="ExternalInput")
    wk4 = nc.dram_tensor("wk4", [P, KT, MG], BF16, kind="ExternalInput")
    wv4 = nc.dram_tensor("wv4", [P, KT, MG], BF16, kind="ExternalInput")
    wo4 = nc.dram_tensor("wo4", [P, MSUB, D_MODEL], BF16, kind="ExternalInput")
    tmask = nc.dram_tensor("tmask", [P, P], BF16, kind="ExternalInput")
    ident2 = nc.dram_tensor("ident2", [P, 2, P], BF16, kind="ExternalInput")
    # output y^T tiled [qt, p, nt, 512]: 4 nt-tiles per DMA -> 4KB rows
    yT4 = nc.dram_tensor("yT4", [QT, P, NT, 512], BF16, kind="ExternalOutput")

    with tile.TileContext(nc) as tc:
        with (
            tc.tile_pool(name="wpool", bufs=1) as wpool,
            tc.tile_pool(name="qkv", bufs=1) as qkv,
            tc.tile_pool(name="xs", bufs=2) as xs,
            tc.tile_pool(name="oh", bufs=2) as ohp,
            tc.tile_pool(name="ys", bufs=4) as ysp,
            tc.tile_pool(name="attn", bufs=3) as attn,
            tc.tile_pool(name="attnc", bufs=1) as attnc,
            tc.tile_pool(name="ps_s", bufs=2, space="PSUM") as ps_s,
            tc.tile_pool(name="ps_o", bufs=1, space="PSUM") as ps_o,
            tc.tile_pool(name="ps_c", bufs=2, space="PSUM") as ps_c,
        ):
            # ---- persistent sbuf ----
            w_sb = {}
            for name in ("q", "k", "v"):
                w_sb[name] = wpool.tile(
                    [P, KT, MG], BF16, tag=f"w{name}", name=f"w{name}"
                )
            wo_sb = wpool.tile([P, MSUB, D_MODEL], BF16, tag="wo")
            qT_sb = qkv.tile([P, MSUB, S], BF16, tag="qT")
            kT_sb = qkv.tile([P, MSUB, S], BF16, tag="kT")
            v_sb = qkv.tile([P, JT, HG, DK + 1], BF16, tag="v")
            nc.vector.memset(v_sb[:, :, :, DK : DK + 1], 1.0)

            tm_sb = attnc.tile([P, P], BF16, tag="tm")
            id2_sb = attnc.tile([P, 2, P], BF16, tag="id2")
            # [33,128] selector: row 0 lights partitions 0:64, row 32 lights
            # 64:128 (partition bases must be 32-aligned, so Z lives in
            # partitions 0 and 32 of z2)
            sel2 = attnc.tile([33, P], F32R, tag="sel2")
            nc.vector.memset(sel2[:].bitcast(F32), 0.0)
            nc.vector.memset(sel2[0:1, 0:DK].bitcast(F32), 1.0)
            nc.vector.memset(sel2[32:33, DK:P].bitcast(F32), 1.0)

            # ---- input DMA: split across queues, first-needed-first ----
            warm_src = attnc.tile([P, 256], BF16, tag="warm_src")
            nc.vector.memset(warm_src[:], 0.5)
            nc.sync.dma_start(tm_sb[:], tmask[:])
            nc.sync.dma_start(id2_sb[:], ident2[:])
            x_tiles = [None] * QT

            def issue_x_dma(st):
                x_tiles[st] = xs.tile([P, KT, 512], BF16, tag="x", name=f"x{st}")
                for kp in range(4):
                    nc.sync.dma_start(
                        x_tiles[st][:, 2 * kp : 2 * kp + 2],
                        xt4[st, :, 2 * kp : 2 * kp + 2],
                    )

            issue_x_dma(0)
            for name, wsrc in (("q", wq4), ("k", wk4), ("v", wv4)):
                nc.sync.dma_start(w_sb[name][:, 0:4], wsrc[:, 0:4])
                nc.sync.dma_start(w_sb[name][:, 4:8], wsrc[:, 4:8])
            nc.sync.dma_start(wo_sb[:], wo4[:])

            # warm the PE while the x/w DMAs land (memset source, no DMA
            # dependency): p-state ramps before the first projection chain
            warm = ps_c.tile([P, 512], F32, tag="pp", name="warm")
            for _ in range(52):
                nc.tensor.matmul(
                    warm[:, 0:256], warm_src[:, 0:P], warm_src[:],
                    start=True, stop=True, skip_group_check=True,
                )

            # ---- filler-step factories (each step = ~4 matmuls on PE) ----
            def proj_qk_steps(name, dst, st):
                ssl = slice(st * 512, (st + 1) * 512)
                w = w_sb[name]
                x_t = x_tiles[st]
                steps = []
                for mt in range(MSUB):
                    msl = slice(mt * P, (mt + 1) * P)
                    holder = {}

                    def s1(mt=mt, msl=msl, holder=holder):
                        pt = ps_c.tile([P, 512], F32, tag="pp", name="prq")
                        holder["pt"] = pt
                        for kt in range(4):
                            nc.tensor.matmul(
                                pt[:], w[:, kt, msl], x_t[:, kt],
                                start=(kt == 0), stop=False,
                            )

                    def s2(mt=mt, msl=msl, holder=holder):
                        pt = holder["pt"]
                        for kt in range(4, KT):
                            nc.tensor.matmul(
                                pt[:], w[:, kt, msl], x_t[:, kt],
                                start=False, stop=(kt == KT - 1),
                            )
                        nc.vector.tensor_copy(dst[:, mt, ssl], pt[:])

                    steps += [s1, s2]
                return steps

            def proj_v_steps(st):
                x_t = x_tiles[st]
                steps = []
                for ssub in range(4):
                    jt = st * 4 + ssub
                    s0 = ssub * P
                    holder = {}

                    def s1(jt=jt, s0=s0, holder=holder):
                        pt = ps_c.tile([P, 512], F32, tag="pp", name="prv")
                        holder["pt"] = pt
                        for kt in range(4):
                            nc.tensor.matmul(
                                pt[:], x_t[:, kt, s0 : s0 + P], w_sb["v"][:, kt],
                                start=(kt == 0), stop=False,
                            )

                    def s2(jt=jt, s0=s0, holder=holder):
                        pt = holder["pt"]
                        for kt in range(4, KT):
                            nc.tensor.matmul(
                                pt[:], x_t[:, kt, s0 : s0 + P], w_sb["v"][:, kt],
                                start=False, stop=(kt == KT - 1),
                            )
                        nc.vector.tensor_copy(
                            v_sb[:, jt, :, 0:DK],
                            pt.rearrange("p (h d) -> p h d", h=HG),
                        )

                    steps += [s1, s2]
                return steps

            def outproj_steps(ohT_prev, qt_prev):
                steps = []
                holder = {}
                for nt in range(NT):
                    def s1(nt=nt):
                        py = ps_c.tile([P, 512], F32, tag="pp", name="py")
                        for mt in range(MSUB):
                            nc.tensor.matmul(
                                py[:],
                                wo_sb[:, mt, nt * P : (nt + 1) * P],
                                ohT_prev[:, mt, :],
                                start=(mt == 0), stop=(mt == MSUB - 1),
                            )
                        if nt % 4 == 0:
                            holder["y4"] = ysp.tile(
                                [P, 4, 512], BF16, tag="y", name="y4"
                            )
                        nc.vector.tensor_copy(holder["y4"][:, nt % 4, :], py[:])
                        if nt % 4 == 3:  # 4 tiles buffered -> one 4KB-row DMA
                            nc.sync.dma_start(
                                yT4[qt_prev, :, nt - 3 : nt + 1], holder["y4"][:]
                            )

                    steps.append(s1)
                return steps

            # ---- attention primitives ----
            def emit_scores(qt, hp, jt):
                jsl = slice(jt * P, (jt + 1) * P)
                di = jt - qt * 4
                delta = 128 * di if di >= 0 else 0
                qsl_d = slice(qt * 512 + delta, (qt + 1) * 512)
                ss = ps_s.tile([P, 2, 512], F32, tag="ss")
                nc.tensor.matmul(
                    ss[:, 0, delta:],
                    kT_sb[0:DK, hp, jsl],
                    qT_sb[0:DK, hp, qsl_d],
                    start=True, stop=False, skip_group_check=True,
                )
                nc.tensor.matmul(
                    ss[:, 1, delta:],
                    kT_sb[DK:P, hp, jsl],
                    qT_sb[DK:P, hp, qsl_d],
                    start=True, stop=(di < 0), skip_group_check=True,
                )
                if di >= 0:
                    nc.tensor.matmul(
                        ss[:, :, delta : delta + P],
                        tm_sb[:], id2_sb[:],
                        start=False, stop=True, skip_group_check=True,
                    )
                return ss, delta

            # ---- main interleaved schedule ----
            # x(st=1) streams while proj(st=0) runs standalone (attention
            # qt=0 depends on proj(st=0))
            issue_x_dma(1)
            for step in proj_qk_steps("q", qT_sb, 0):
                step()
            for step in proj_qk_steps("k", kT_sb, 0):
                step()
            for step in proj_v_steps(0):
                step()

            prev = None  # (ohT, qt) with outproj pending
            deferred_kv3 = None
            fin = {}
            for qt in range(QT):
                # x DMA two q-tiles ahead (xs bufs=2: the tile waits for the
                # previous generation's readers automatically)
                if qt + 2 < QT:
                    issue_x_dma(qt + 2)
                fillers = []
                nfront = 0
                if qt + 1 < QT:
                    st = qt + 1
                    fillers += proj_qk_steps("q", qT_sb, st)
                    if st < QT - 1:
                        fillers += proj_qk_steps("k", kT_sb, st)
                        fillers += proj_v_steps(st)
                    else:
                        # defer k/v(st=3) into qt=3's early blocks: qt3 is
                        # ACT(exp)-bound, so this PE work fills its bubbles
                        deferred_kv3 = (
                            proj_qk_steps("k", kT_sb, st) + proj_v_steps(st)
                        )
                if qt == QT - 1 and deferred_kv3 is not None:
                    fillers = deferred_kv3 + fillers
                    # k/v(st3) feeds this qt's jt>=12 blocks: hp0 hits jt=12
                    # at block 12, so these steps must front-load
                    nfront = len(deferred_kv3)
                if prev is not None:
                    fillers += outproj_steps(prev[0], prev[1])

                njt = 4 * (qt + 1)
                ohT = ohp.tile([P, MSUB, 512], BF16, tag="ohT")
                work = [(hp, jt) for hp in range(MSUB) for jt in range(njt)]
                nw = len(work)
                nf = len(fillers)
                fi = 0
                po = {}
                pend = emit_scores(qt, 0, 0)
                for wi, (hp, jt) in enumerate(work):
                    if jt == 0:
                        po[hp] = (
                            ps_o.tile([DK + 1, 512], F32, tag="poA", name="po_a"),
                            ps_o.tile([DK + 1, 512], F32, tag="poB", name="po_b"),
                        )
                    ss, delta = pend
                    pp = attn.tile([P, 2, 512], BF16, tag="pp")
                    nc.scalar.activation(
                        pp[:, :, delta:], ss[:, :, delta:], EXP, scale=0.125
                    )
                    # next block's scores ahead of this block's PV
                    if wi + 1 < nw:
                        pend = emit_scores(qt, *work[wi + 1])
                    # filler quota for this block (front-loaded steps must
                    # finish within the first 11 blocks)
                    want = (wi + 1) * nf // nw
                    if nfront and wi < 12:
                        want = max(want, min(nfront, (wi + 1) * nfront // 11))
                    while fi < want:
                        fillers[fi]()
                        fi += 1
                    po_a, po_b = po[hp]
                    nc.tensor.matmul(
                        po_a[:, delta:],
                        v_sb[:, jt, 2 * hp, :],
                        pp[:, 0, delta:],
                        start=(jt == 0), stop=(jt == njt - 1),
                    )
                    nc.tensor.matmul(
                        po_b[:, delta:],
                        v_sb[:, jt, 2 * hp + 1, :],
                        pp[:, 1, delta:],
                        start=(jt == 0), stop=(jt == njt - 1),
                    )
                    if jt == njt - 1:
                        z2 = attnc.tile([33, 512], F32R, tag="z2")
                        if hp == 0 and qt == 0:
                            # rows 1..31 are never written; zero once so the
                            # K=33 broadcast matmul multiplies 0 * 0
                            nc.vector.memset(z2[:].bitcast(F32), 0.0)
                        dst = ohT[:, hp, :]
                        nc.vector.tensor_copy(z2[0:1, :], po_a[DK : DK + 1, :])
                        nc.vector.tensor_copy(z2[32:33, :], po_b[DK : DK + 1, :])
                        nc.vector.tensor_copy(dst[0:DK], po_a[0:DK, :])
                        nc.vector.tensor_copy(dst[DK:P], po_b[0:DK, :])
                        last_ep = qt == QT - 1 and hp == MSUB - 1
                        if last_ep:
                            # very last epilogue: pre-open the first final
                            # outproj chain (mt 0..2 only need already-scaled
                            # heads) so the PE covers the z2->bcz->mult chain
                            fin["py0"] = fin_py0 = ps_c.tile(
                                [P, 512], F32, tag="pp", name="py0f"
                            )
                            for mt in range(MSUB - 1):
                                nc.tensor.matmul(
                                    fin_py0[:],
                                    wo_sb[:, mt, 0:P],
                                    ohT[:, mt, :],
                                    start=(mt == 0), stop=False,
                                )
                        # keep the PE busy while the DVE drains z2 (the bcz
                        # matmul below would otherwise head-of-line block)
                        for _ in range(2):
                            if fi < nf:
                                fillers[fi]()
                                fi += 1
                        bcz = ps_c.tile([P, 512], F32, tag="pp", name="bcz")
                        nc.tensor.matmul(
                            bcz[:], sel2[:], z2[:], start=True, stop=True
                        )
                        bcr = attnc.tile([P, 512], F32, tag="bcr")
                        nc.vector.reciprocal_approx_fast(bcr[:], bcz[:])
                        nc.gpsimd.tensor_tensor(
                            dst, dst, bcr[:], mybir.AluOpType.mult
                        )
                        if last_ep:
                            nc.tensor.matmul(
                                fin_py0[:],
                                wo_sb[:, MSUB - 1, 0:P],
                                ohT[:, MSUB - 1, :],
                                start=False, stop=True,
                            )
                while fi < len(fillers):
                    fillers[fi]()
                    fi += 1
                prev = (ohT, qt)
            # final output projection: nt=0 was computed inside the last
            # epilogue; evict it and run nt 1..7
            ohT3, qt3 = prev
            y2f = None
            for nt in range(NT):
                if nt % 2 == 0:
                    y2f = ysp.tile([P, 2, 512], BF16, tag="y2", name="y2f")
                if nt == 0:
                    py = fin["py0"]
                else:
                    py = ps_c.tile([P, 512], F32, tag="pp", name="pyf")
                    for mt in range(MSUB):
                        nc.tensor.matmul(
                            py[:],
                            wo_sb[:, mt, nt * P : (nt + 1) * P],
                            ohT3[:, mt, :],
                            start=(mt == 0), stop=(mt == MSUB - 1),
                        )
                # alternate eviction engines so chains never wait on one
                if nt % 2 == 0:
                    nc.vector.tensor_copy(y2f[:, 0, :], py[:])
                else:
                    nc.scalar.copy(y2f[:, 1, :], py[:])
                    nc.sync.dma_start(yT4[qt3, :, nt - 1 : nt + 1], y2f[:])

    nc.finalize()
    return nc


def _get_nc() -> bass.Bass:
    global _CACHED_NC
    if _CACHED_NC is None:
        _CACHED_NC = build_nc()
    return _CACHED_NC


def _make_masks() -> np.ndarray:
    import ml_dtypes

    k = np.arange(P)[:, None]
    j = np.arange(P)[None, :]
    return np.where(j > k, -30000.0, 0.0).astype(ml_dtypes.bfloat16)


def _make_ident2() -> np.ndarray:
    import ml_dtypes

    e = np.eye(P, dtype=np.float32)
    return np.stack([e, e], axis=1).astype(ml_dtypes.bfloat16)


def make_in_maps(inputs):
    import ml_dtypes

    bf = ml_dtypes.bfloat16
    x = np.asarray(inputs["x"], np.float32)
    q_heads = np.asarray(inputs["q_heads"], np.float32)
    k_heads = np.asarray(inputs["k_heads"], np.float32)
    v_heads = np.asarray(inputs["v_heads"], np.float32)
    output_proj = np.asarray(inputs["output_proj"], np.float32)

    tm = _make_masks()
    id2 = _make_ident2()

    def tile_w(wT):  # [1024, 512] -> [p, kt, m]
        return np.ascontiguousarray(
            wT.reshape(KT, P, MG).transpose(1, 0, 2)
        ).astype(bf)

    in_maps = []
    for core in range(N_CORES):
        b, g = divmod(core, 2)
        gsl = slice(g * MG, (g + 1) * MG)
        xT = x[b].T  # [1024, 2048]
        xt4 = np.ascontiguousarray(
            xT.reshape(KT, P, QT, 512).transpose(2, 1, 0, 3)
        ).astype(bf)  # [st, p, kt, 512]
        wo = output_proj[:, gsl].T  # [512, 1024]
        wo4 = np.ascontiguousarray(
            wo.reshape(MSUB, P, D_MODEL).transpose(1, 0, 2)
        ).astype(bf)
        in_maps.append(
            {
                "xt4": xt4,
                "wq4": tile_w(q_heads[gsl].T),
                "wk4": tile_w(k_heads[gsl].T),
                "wv4": tile_w(v_heads[gsl].T),
                "wo4": wo4,
                "tmask": tm,
                "ident2": id2,
            }
        )
    return in_maps


def kernel(x, q_heads, k_heads, v_heads, output_proj):
    inputs = {
        "x": x,
        "q_heads": q_heads,
        "k_heads": k_heads,
        "v_heads": v_heads,
        "output_proj": output_proj,
    }
    in_maps = make_in_maps(inputs)
    nc = _get_nc()
    res = run_bass_kernel_spmd(nc, in_maps, list(range(N_CORES)))
    y = np.empty((B, S, D_MODEL), np.float32)
    for b in range(B):
        # yT4 [qt, p, nt, 512] -> yT [nt*128+p, qt*512+c]
        acc = res.results[2 * b]["yT4"].astype(np.float32) + res.results[
            2 * b + 1
        ]["yT4"].astype(np.float32)
        yT = acc.transpose(2, 1, 0, 3).reshape(D_MODEL, S)
        y[b] = yT.T
    return y
